# revision 2
# baseline (speedup 1.0000x reference)
"""Trainium2 Bass kernel for nn_AttentionModel_47983374631276.

SDPA attention: B=2, H=16, S=2048, D=128, fp8-representable q/k/v with
per-tensor dequant scales (qs, ks, vs).

Sharding: batch*heads = 32 pairs -> 4 heads per core across 8 cores.
Each core runs its full S x S attention locally; no cross-core comm.

Per-head device algorithm (v2 — fp8 DoubleRow mm1 + multi-engine exp):
  1. matmul1 in fp8e4 DoubleRow perf mode (lossless: q/k values are
     exactly fp8): contraction d=128 split as two 64-partition tiles,
     lhsT = K^T [64,2,128cols], rhs = Q^T [64,2,512], PSUM f32 out
     [128,512] at 0.5 cycles/row — 2x the bf16 rate.
  2. exp split across engines by chunk (all compute the same e^(c*L),
     so any per-chunk mix is consistent; no shift needed: |c*L| < 6
     so P' spans ~2^-9..2^6, comfortably inside fp16):
       'S' chunks: ScalarE ACT Exp (scale=c_nat) PSUM->fp16 SBUF.
       'B' chunks: DVE+Pool Schraudolph pipeline:
            DVE: y=i16(L*c2*1024+15360); g=(y&0x3FF)|0x3C00 (fp16 bits
                 of 1+f); t1=qc2*g+qc1; t2=t1*g
            Pool: t3=t2+qc0; P'=t3*u  (u = y bitcast fp16 = 2^n(1+f))
            The quadratic qc(g) ~ 2^(g-1)/g corrects the Schraudolph
            piecewise-linear error to ~1e-2 max / 2.4e-3 rms.
       'A' chunks: same chain entirely on DVE.
  3. matmul2 (fp16): out_ext[q, 129] = sum_k P'^T[k,q].T @ [V | 1]
     (ones column yields the softmax denominator for free). fp8 P'
     here would breach the 2e-2 gate (measured 2.8e-2), so fp16.
  4. evac: out[q, :128] * vs / out[q, 128] on DVE, DMA to DRAM.

Software pipelining: phase2 of head h-1 is emitted AFTER phase1 of head
h so matmul2 fills TensorE gaps while the exp engines chew on phase1.

PSUM budget (8 banks): ps1 chunks [128,3,512] x2 bufs = 6 banks,
ps2 accumulators [128,129] x2 bufs = 2 banks.
"""

import math
import os

import numpy as np
import ml_dtypes

import concourse.bacc as bacc
import concourse.bass as bass
import concourse.tile as tile
import concourse.mybir as mybir
from concourse.bass_utils import run_bass_kernel_spmd

N_CORES = 8
HEADS_PER_CORE = 4
S = 2048
D = 128
P = 128            # partitions
KT = S // P        # 16 k tiles per head
QQ = 4             # q chunks of 512 for matmul1
QW = S // QQ       # 512
HALF_SLICES = 2 * KT  # 32 slices (qq_loc, kt) per half

BF16 = mybir.dt.bfloat16
FP16 = mybir.dt.float16
FP8 = mybir.dt.float8e4
I16 = mybir.dt.int16
F32 = mybir.dt.float32

LOG2E = math.log2(math.e)
# minimax quadratic fit of 2^(g-1)/g on [1,2]
QC2, QC1, QC0 = 0.21568139, -0.63294407, 1.40770541

# Per-half chunk pattern: (engine, n_slices). Sums to 32.
#  'S' = ScalarE ACT exp; 'B' = DVE front + Pool tail; 'A' = all-DVE.
# Balance target ~24.5us/head on each of ScalarE/DVE/Pool.
PATTERN = [
    ("S", 3), ("S", 3), ("B", 3), ("S", 3), ("S", 3),
    ("B", 2), ("S", 3), ("S", 3), ("B", 2), ("S", 3),
    ("S", 3), ("A", 1),
]
assert sum(n for _, n in PATTERN) == HALF_SLICES

# Stash of the most recent run results / program for test harnesses.
LAST_RESULTS = None
LAST_NC = None


def _build_program(c_nat: float, vs_val: float, repeat: int = 1):
    nc = bacc.Bacc()

    qT_d = nc.dram_tensor("qT", [HEADS_PER_CORE, 64, 2, S], FP8, kind="ExternalInput")
    kT_d = nc.dram_tensor("kT", [HEADS_PER_CORE, 64, 2, S], FP8, kind="ExternalInput")
    v_d = nc.dram_tensor("v", [HEADS_PER_CORE, S, D], FP16, kind="ExternalInput")
    out_d = nc.dram_tensor("out", [HEADS_PER_CORE, S, D], F32, kind="ExternalOutput")

    cvt_scale = float(c_nat * LOG2E * 1024.0)

    with tile.TileContext(nc) as tc:
        with (
            tc.tile_pool(name="io", bufs=2) as io_pool,
            tc.tile_pool(name="ptp", bufs=4) as pt_pool,
            tc.tile_pool(name="chain", bufs=3) as ch_pool,
            tc.tile_pool(name="outp", bufs=4) as out_pool,
            tc.tile_pool(name="smallp", bufs=4) as small_pool,
            tc.tile_pool(name="ps1p", bufs=2, space="PSUM") as ps1_pool,
            tc.tile_pool(name="ps2p", bufs=2, space="PSUM") as ps2_pool,
        ):

            P1_BAND = 0
            P2_BAND = 10_000_000
            HEAD_STRIDE = 100_000

            def emit_load(h, step=None):
                tc.cur_priority = P1_BAND + (h if step is None else step) * HEAD_STRIDE
                kT_sb = io_pool.tile([64, 2, S], FP8, tag="kT")
                qT_sb = io_pool.tile([64, 2, S], FP8, tag="qT")
                if h == 0:
                    # First head: small leading blocks so the first chunk's
                    # matmuls depend on minimal DMA.
                    nc.sync.dma_start(kT_sb[:, :, : 3 * P], kT_d[h, :, :, : 3 * P])
                    nc.gpsimd.dma_start(qT_sb[:, :, :QW], qT_d[h, :, :, :QW])
                    nc.sync.dma_start(kT_sb[:, :, 3 * P : QW], kT_d[h, :, :, 3 * P : QW])
                    for b in range(1, QQ):
                        sl = slice(b * QW, (b + 1) * QW)
                        nc.sync.dma_start(kT_sb[:, :, sl], kT_d[h, :, :, sl])
                        nc.gpsimd.dma_start(qT_sb[:, :, sl], qT_d[h, :, :, sl])
                else:
                    for b in range(QQ):
                        sl = slice(b * QW, (b + 1) * QW)
                        nc.sync.dma_start(kT_sb[:, :, sl], kT_d[h, :, :, sl])
                        nc.gpsimd.dma_start(qT_sb[:, :, sl], qT_d[h, :, :, sl])
                v_sb = io_pool.tile([P, KT, D + 1], FP16, tag="v")
                nc.sync.dma_start(
                    v_sb[:, :, :D], v_d[h].rearrange("(t p) d -> p t d", p=P)
                )
                nc.vector.memset(v_sb[:, :, D : D + 1], 1.0)
                return qT_sb, kT_sb, v_sb

            def emit_chunk_S(pth, ps1, s0, n):
                nc.scalar.activation(
                    pth[:, s0 : s0 + n, :],
                    ps1[:, :n, :],
                    mybir.ActivationFunctionType.Exp,
                    scale=c_nat,
                )

            def emit_chunk_BA(pth, ps1, s0, n, tail_engine):
                # DVE front: convert + mask + quadratic start
                y = ch_pool.tile([P, 3, QW], I16, tag="y")
                nc.vector.tensor_scalar(
                    y[:, :n, :], ps1[:, :n, :], cvt_scale, 15360.0,
                    mybir.AluOpType.mult, mybir.AluOpType.add,
                )
                u = y.bitcast(FP16)
                gi = ch_pool.tile([P, 3, QW], I16, tag="gi")
                nc.vector.tensor_scalar(
                    gi[:, :n, :], y[:, :n, :], 0x03FF, 0x3C00,
                    mybir.AluOpType.bitwise_and, mybir.AluOpType.bitwise_or,
                )
                g = gi.bitcast(FP16)
                t1 = ch_pool.tile([P, 3, QW], FP16, tag="t1")
                nc.vector.tensor_scalar(
                    t1[:, :n, :], g[:, :n, :], QC2, QC1,
                    mybir.AluOpType.mult, mybir.AluOpType.add,
                )
                t2 = ch_pool.tile([P, 3, QW], FP16, tag="t2")
                nc.vector.tensor_tensor(
                    t2[:, :n, :], t1[:, :n, :], g[:, :n, :], mybir.AluOpType.mult
                )
                eng = nc.gpsimd if tail_engine == "P" else nc.vector
                t3 = ch_pool.tile([P, 3, QW], FP16, tag="t3")
                eng.tensor_scalar(
                    t3[:, :n, :], t2[:, :n, :], 1.0, QC0,
                    mybir.AluOpType.mult, mybir.AluOpType.add,
                )
                eng.tensor_tensor(
                    pth[:, s0 : s0 + n, :], t3[:, :n, :], u[:, :n, :],
                    mybir.AluOpType.mult,
                )

            def emit_phase1(h, qT_sb, kT_sb, step=None):
                tc.cur_priority = P1_BAND + (h if step is None else step) * HEAD_STRIDE + 1000
                halves = []
                for hh in range(2):
                    pth = pt_pool.tile([P, 2 * KT, QW], FP16, tag="pth")
                    halves.append(pth)
                    s0 = 0
                    for eng_t, n in PATTERN:
                        ps1 = ps1_pool.tile([P, 3, QW], F32, tag="ps1")
                        for j in range(n):
                            qq_loc, kt = divmod(s0 + j, KT)
                            nc.tensor.matmul(
                                ps1[:, j, :],
                                lhsT=kT_sb[:, :, kt * P : (kt + 1) * P],
                                rhs=qT_sb[
                                    :, :,
                                    (2 * hh + qq_loc) * QW : (2 * hh + qq_loc + 1) * QW,
                                ],
                                start=True,
                                stop=True,
                                perf_mode=mybir.MatmulPerfMode.DoubleRow,
                            )
                        if eng_t == "S":
                            emit_chunk_S(pth, ps1, s0, n)
                        elif eng_t == "B":
                            emit_chunk_BA(pth, ps1, s0, n, "P")
                        else:
                            emit_chunk_BA(pth, ps1, s0, n, "V")
                        s0 += n
                return halves

            def emit_phase2(h, halves, v_sb, step=None, tail=False):
                tc.cur_priority = P2_BAND + (h if step is None else step) * HEAD_STRIDE
                o2_sb = None
                for qt in range(KT):
                    pth = halves[qt // (2 * QQ)]
                    qq_loc, qcol = divmod(qt % (2 * QQ), QQ)
                    if tail and qt >= KT - 2:
                        # last head: phase1 psum slots are dead -- recycle
                        big = ps1_pool.tile([P, 3, QW], F32, tag="ps1")
                        ps2 = big[:, 0, : D + 1]
                    else:
                        ps2 = ps2_pool.tile([P, D + 1], F32, tag="ps2")
                    for kt in range(KT):
                        nc.tensor.matmul(
                            ps2,
                            lhsT=pth[:, qq_loc * KT + kt, qcol * P : (qcol + 1) * P],
                            rhs=v_sb[:, kt, :],
                            start=(kt == 0),
                            stop=(kt == KT - 1),
                        )
                    recip = small_pool.tile([P, 1], F32, tag="recip")
                    nc.vector.reciprocal(recip, ps2[:, D : D + 1])
                    if tail and qt >= KT - 2:
                        if o2_sb is None:
                            o2_sb = out_pool.tile([P, 2, D], F32, tag="o2")
                        o_sb = o2_sb[:, qt - (KT - 2), :]
                    else:
                        o_sb = out_pool.tile([P, D], F32, tag="o")
                    nc.vector.tensor_scalar(
                        o_sb,
                        ps2[:, :D],
                        recip,
                        vs_val,
                        mybir.AluOpType.mult,
                        mybir.AluOpType.mult,
                    )
                    if tail and qt == KT - 1:
                        nc.scalar.dma_start(
                            out_d[h, (KT - 2) * P : KT * P, :].rearrange(
                                "(j p) d -> p j d", p=P
                            ),
                            o2_sb,
                        )
                    elif not (tail and qt == KT - 2):
                        nc.sync.dma_start(out_d[h, qt * P : (qt + 1) * P, :], o_sb)

            prev = None
            for step in range(HEADS_PER_CORE * repeat):
                h = step % HEADS_PER_CORE
                qT_sb, kT_sb, v_sb = emit_load(h, step)
                halves = emit_phase1(h, qT_sb, kT_sb, step)
                if prev is not None:
                    emit_phase2(*prev)
                prev = (h, halves, v_sb, step)
            emit_phase2(*prev, tail=True)

    nc.compile()
    return nc


def kernel(s, q, k, v, qs, ks, vs):
    global LAST_RESULTS, LAST_NC
    q = np.asarray(q, dtype=np.float32)
    k = np.asarray(k, dtype=np.float32)
    v = np.asarray(v, dtype=np.float32)
    qs = np.asarray(qs, dtype=np.float32)
    ks = np.asarray(ks, dtype=np.float32)
    vs = np.asarray(vs, dtype=np.float32)

    B, H, S_, D_ = q.shape
    assert (S_, D_) == (S, D) and B * H == N_CORES * HEADS_PER_CORE

    # DoubleRow layout: [head, 64, 2, S] where tile i holds d in
    # [64*i, 64*i+64). q/k values are fp8-representable -> cast lossless
    # (up to e4m3fn/fnuz subnormal edge, well below tolerance).
    def pack_dr(x):
        # x: [BH, S, D] -> [BH, 64, 2, S]
        xt = x.reshape(B * H, S, 2, 64).transpose(0, 3, 2, 1)
        return np.ascontiguousarray(xt).astype(ml_dtypes.float8_e4m3)

    qT8 = pack_dr(q.reshape(B * H, S, D))
    kT8 = pack_dr(k.reshape(B * H, S, D))
    vb = np.ascontiguousarray(v.reshape(B * H, S, D)).astype(np.float16)

    c_nat = float(
        np.float32(qs[0]) * np.float32(ks[0]) * np.float32(1.0 / math.sqrt(D))
    )
    vs_val = float(vs[0])
    # No logit shift: with sigma_logit = qs*ks the extreme |c*L| stays
    # well inside fp16 exp range. Guard against pathological scales.
    assert c_nat * 5.8 * math.sqrt(D) * 1.6 < 11.0, "logit shift needed"

    nc = _build_program(c_nat, vs_val)
    LAST_NC = nc

    in_maps = []
    for c in range(N_CORES):
        lo, hi = c * HEADS_PER_CORE, (c + 1) * HEADS_PER_CORE
        in_maps.append(
            {
                "qT": np.ascontiguousarray(qT8[lo:hi]),
                "kT": np.ascontiguousarray(kT8[lo:hi]),
                "v": np.ascontiguousarray(vb[lo:hi]),
            }
        )

    try:
        res = run_bass_kernel_spmd(nc, in_maps, core_ids=list(range(N_CORES)))
    except ModuleNotFoundError:
        os.environ["BASS_NEVER_TRACE"] = "1"
        res = run_bass_kernel_spmd(nc, in_maps, core_ids=list(range(N_CORES)))
    LAST_RESULTS = res

    out = np.stack([r["out"] for r in res.results])  # [8, 4, S, D] f32
    return out.reshape(B, H, S, D).astype(np.float32)
